# revision 1
# baseline (speedup 1.0000x reference)
"""GNN message-passing (nn_Cell) kernel for Trainium2, 8-core SPMD Bass.

Strategy: destination-sharded SpMM. The 20000 nodes are split 2500/core
(padded to 2560 = 20 tiles x 128 rows). Per (core, adjacency), edges are
bucketed by destination tile and padded to a uniform T chunks of 128 edges
per tile. Each of the 10 (source-state, adjacency) segment-sums runs as:

  dma_gather   : pull the 256-float source rows for 128*T edges from the
                 replicated state table in HBM (one edge per partition)
  DVE          : build one-hot MT[e, d] = (iota[d] == dst_e) * val_e per
                 128-edge chunk (iota kept in PSUM so the op runs in 1x
                 mode and never locks the SBUF port SWDGE needs)
  TensorE      : psum[128 dst, 256] += MT.T @ G_chunk   (PSUM accumulate)

Weighted PSUM drains (ACT scale + DVE add) feed three persistent SBUF
accumulators (s1 / s2 / out). An AllGather shares each newly computed state
across the 8 cores between rounds; the adjacency-3 pass of each round is
scheduled after the AllGather so it hides the collective latency. Final
LayerNorm + exact GELU run fused on DVE/ACT.

State tables are stored as float32r (TF32-like), giving ~1e-4 relative
error while the one-hot matmuls run at full PE rate.
"""
import sys

for _p in ("/opt/trn_rl_repo",):
    if _p not in sys.path:
        sys.path.insert(0, _p)

import numpy as np
import concourse.bacc as bacc
import concourse.tile as tile
from concourse import mybir
from concourse.bass_utils import run_bass_kernel_spmd

N_CORES = 8
N_NODES = 20000
N_ADJ = 4
D = 256
ROWS_PER = N_NODES // N_CORES          # 2500
N_TILES = (ROWS_PER + 127) // 128      # 20
PAD_ROWS = N_TILES * 128               # 2560
PADN = PAD_ROWS * N_CORES              # 20480
LN_EPS = 1e-5

MODE = "f32r"                          # "f32" | "f32r" | "bf16"
DT_MAP = {
    "f32": mybir.dt.float32,
    "f32r": mybir.dt.float32r,
    "bf16": mybir.dt.bfloat16,
}

LAST_BUILD = {}


def balance_rows(rows):
    """Greedy assignment of nodes to (core, tile, slot) balancing the four
    per-adjacency in-degree loads across all 160 destination tiles, so every
    tile needs the same (minimal) number of 128-edge chunks."""
    deg = np.stack([np.bincount(rows[a], minlength=N_NODES)
                    for a in range(N_ADJ)], 1)
    order = np.argsort(-deg.sum(1))
    NB = N_CORES * N_TILES
    load = np.zeros((NB, N_ADJ), dtype=np.int64)
    bcount = np.zeros(NB, dtype=np.int32)
    ccount = np.zeros(N_CORES, dtype=np.int32)
    core_of = np.zeros(N_NODES, np.int32)
    tile_of = np.zeros(N_NODES, np.int32)
    slot_of = np.zeros(N_NODES, np.int32)
    bucket_core = np.arange(NB) // N_TILES
    BIG = 1 << 60
    for n in order:
        dn = deg[n]
        score = (load + dn).max(1)
        score[(bcount >= 128) | (ccount[bucket_core] >= ROWS_PER)] = BIG
        b = int(np.argmin(score))
        load[b] += dn
        core_of[n] = bucket_core[b]
        tile_of[n] = b % N_TILES
        slot_of[n] = bcount[b]
        bcount[b] += 1
        ccount[bucket_core[b]] += 1
    return core_of, tile_of, slot_of


def preprocess(x, rows, cols, vals):
    """Host-side edge partitioning. Returns per-core input maps, T, and the
    node -> padded-row permutation."""
    rows = np.asarray(rows).astype(np.int64)
    cols = np.asarray(cols).astype(np.int64)
    vals = np.asarray(vals).astype(np.float32)
    x = np.asarray(x).astype(np.float32)

    core_of, tile_of, slot_of = balance_rows(rows)
    padded_id = core_of * PAD_ROWS + tile_of * 128 + slot_of

    core = core_of[rows]
    tl = tile_of[rows]
    dst = slot_of[rows].astype(np.float32)
    gcol = padded_id[cols]

    T = []
    for a in range(N_ADJ):
        key = core[a] * N_TILES + tl[a]
        cnt = np.bincount(key, minlength=N_CORES * N_TILES)
        T.append(int(np.ceil(cnt.max() / 128)))

    idx_np = [np.zeros((N_CORES, 128, N_TILES * T[a] * 8), dtype=np.int16)
              for a in range(N_ADJ)]
    dst_np = [np.zeros((N_CORES, 128, N_TILES * T[a]), dtype=np.float32)
              for a in range(N_ADJ)]
    val_np = [np.zeros((N_CORES, 128, N_TILES * T[a]), dtype=np.float32)
              for a in range(N_ADJ)]
    for a in range(N_ADJ):
        Ta = T[a]
        key = core[a] * N_TILES + tl[a]
        order = np.argsort(key, kind='stable')
        key_s = key[order]
        gcol_s = gcol[a][order].astype(np.int16)
        dst_s = dst[a][order]
        val_s = vals[a][order]
        bound = np.searchsorted(key_s, np.arange(N_CORES * N_TILES))
        j = np.arange(len(key_s)) - bound[key_s]
        c_arr = key_s // N_TILES
        t_arr = key_s % N_TILES
        idx_np[a][c_arr, j % 16, t_arr * Ta * 8 + j // 16] = gcol_s
        dst_np[a][c_arr, j % 128, t_arr * Ta + j // 128] = dst_s
        val_np[a][c_arr, j % 128, t_arr * Ta + j // 128] = val_s
        for r in range(1, 8):
            idx_np[a][:, 16 * r:16 * (r + 1), :] = idx_np[a][:, :16, :]

    xT = np.zeros((N_CORES, D, PAD_ROWS), dtype=np.float32)
    xT[core_of, :, tile_of * 128 + slot_of] = x

    in_maps = []
    for c in range(N_CORES):
        m = {"xT_in": xT[c]}
        for a in range(N_ADJ):
            m[f"idx{a}_in"] = idx_np[a][c]
            m[f"dst{a}_in"] = dst_np[a][c]
            m[f"val{a}_in"] = val_np[a][c]
        in_maps.append(m)
    return in_maps, T, padded_id


def make_weights(ws_seq_0, ws_seq_1, ws_res_0, ws_res_1):
    ws_seq_0 = np.asarray(ws_seq_0, dtype=np.float64)
    ws_seq_1 = np.asarray(ws_seq_1, dtype=np.float64)
    ws_res_0 = np.asarray(ws_res_0, dtype=np.float64)
    ws_res_1 = np.asarray(ws_res_1, dtype=np.float64)
    return {
        "wA": {a: ws_seq_0[0][a] / 3 for a in range(3)},
        "wB1": {a: ws_res_0[0][a] / 4 for a in range(4)},
        "wC1": {0: ws_res_1[0][0] / 3, 1: ws_res_1[0][1] / 3,
                3: ws_res_1[0][2] / 3},
        "wB2": {a: ws_seq_0[1][a] / 3 for a in range(3)},
        "wC2": {0: ws_res_1[1][0] / 3, 1: ws_res_1[1][1] / 3,
                3: ws_res_1[1][2] / 3},
        "wC3": {0: ws_seq_1[0] / 2, 1: ws_seq_1[1] / 2},
    }


def build_program(T, weights, mode=MODE):
    tdt = DT_MAP[mode]
    nc = bacc.Bacc("TRN2", target_bir_lowering=False, debug=False,
                   num_devices=N_CORES, num_swdge_queues=2)

    xT_in = nc.dram_tensor("xT_in", [D, PAD_ROWS], mybir.dt.float32,
                           kind="ExternalInput")
    W_in = nc.dram_tensor("W_in", [D, D], mybir.dt.float32,
                          kind="ExternalInput")
    bb_in = nc.dram_tensor("bb_in", [128, D], mybir.dt.float32,
                           kind="ExternalInput")
    iota_in = nc.dram_tensor("iota_in", [128, 128], mybir.dt.float32,
                             kind="ExternalInput")
    idx_in, dstv_in, valv_in = [], [], []
    for a in range(N_ADJ):
        idx_in.append(nc.dram_tensor(f"idx{a}_in", [128, N_TILES * T[a] * 8],
                                     mybir.dt.int16, kind="ExternalInput"))
        dstv_in.append(nc.dram_tensor(f"dst{a}_in", [128, N_TILES * T[a]],
                                      mybir.dt.float32, kind="ExternalInput"))
        valv_in.append(nc.dram_tensor(f"val{a}_in", [128, N_TILES * T[a]],
                                      mybir.dt.float32, kind="ExternalInput"))
    y_out = nc.dram_tensor("y_out", [PAD_ROWS, D], mybir.dt.float32,
                           kind="ExternalOutput")

    rg = [list(range(N_CORES))]
    AF = mybir.ActivationFunctionType
    OP = mybir.AluOpType

    with tile.TileContext(nc) as tc:
        with tc.tile_pool(name="const", bufs=1) as cpool, \
             tc.tile_pool(name="work", bufs=1) as wpool, \
             tc.tile_pool(name="psum", bufs=4, space="PSUM") as ppool, \
             tc.tile_pool(name="dram", bufs=1, space="DRAM") as dpool:

            W_sb = cpool.tile([128, 2, D], mybir.dt.float32, name="W_sb")
            nc.sync.dma_start(W_sb[:],
                              W_in.ap().rearrange("(k p) d -> p k d", p=128))
            bb = cpool.tile([128, D], mybir.dt.float32, name="bb")
            nc.sync.dma_start(bb[:], bb_in[:])
            iota_sb = cpool.tile([128, 128], mybir.dt.float32, name="iota_sb")
            nc.sync.dma_start(iota_sb[:], iota_in[:])
            # iota lives in PSUM: forces the MT-build tensor_scalar into 1x
            # mode so DVE never takes the SBUF port SWDGE (dma_gather
            # descriptor generation) depends on.
            iota = ppool.tile([128, 128], mybir.dt.float32, tag="iota_ps",
                              bufs=1, name="iota_ps")
            nc.vector.tensor_copy(iota[:], iota_sb[:])

            idx_sb, dst_sb, val_sb = [], [], []
            for a in range(N_ADJ):
                t_ = cpool.tile([128, N_TILES * T[a] * 8], mybir.dt.int16,
                                name=f"idx_sb{a}")
                nc.sync.dma_start(t_[:], idx_in[a][:])
                idx_sb.append(t_)
                t_ = cpool.tile([128, N_TILES * T[a]], mybir.dt.float32,
                                name=f"dst_sb{a}")
                nc.sync.dma_start(t_[:], dstv_in[a][:])
                dst_sb.append(t_)
                t_ = cpool.tile([128, N_TILES * T[a]], mybir.dt.float32,
                                name=f"val_sb{a}")
                nc.sync.dma_start(t_[:], valv_in[a][:])
                val_sb.append(t_)

            accA = cpool.tile([128, N_TILES, D], mybir.dt.float32, name="accA")
            accB = cpool.tile([128, N_TILES, D], mybir.dt.float32, name="accB")
            accC = cpool.tile([128, N_TILES, D], mybir.dt.float32, name="accC")
            acc = {"A": accA, "B": accB, "C": accC}
            acc_written = {k: set() for k in acc}

            slice_t, full_t = {}, {}
            for s in ("s0", "s1", "s2"):
                slice_t[s] = dpool.tile([PAD_ROWS, D], tdt, name=f"{s}_slice")
                full_t[s] = dpool.tile([PADN, D], tdt, name=f"{s}_full",
                                       addr_space="Shared")

            # ---- affine: s0 = x @ W + b (own rows only) ----
            for t in range(N_TILES):
                xk = wpool.tile([128, 2, 128], mybir.dt.float32, tag="xk",
                                bufs=3, name="xk")
                nc.sync.dma_start(
                    xk[:], xT_in.ap().rearrange("(k p) r -> p k r", p=128)
                    [:, :, t * 128:(t + 1) * 128])
                ps = ppool.tile([128, D], mybir.dt.float32, tag="ps", bufs=6,
                                name="ps_aff")
                for k in range(2):
                    nc.tensor.matmul(ps[:], xk[:, k, :], W_sb[:, k, :],
                                     start=(k == 0), stop=(k == 1))
                st = wpool.tile([128, D], tdt, tag="stage", bufs=4,
                                name="st_aff")
                nc.vector.tensor_tensor(st[:], ps[:], bb[:], op=OP.add)
                nc.sync.dma_start(slice_t["s0"][t * 128:(t + 1) * 128, :],
                                  st[:])
            nc.gpsimd.collective_compute(
                "AllGather", OP.bypass, replica_groups=rg,
                ins=[slice_t["s0"].opt()], outs=[full_t["s0"].opt()])

            qsel = [0]

            def seg_pass(src_full, a, drains):
                Ta = T[a]
                for t in range(N_TILES):
                    G = wpool.tile([128, max(T), D], tdt, tag="G", bufs=4,
                                   name="G")[:, :Ta, :]
                    qsel[0] ^= 1
                    nc.gpsimd.dma_gather(
                        G, src_full[:],
                        idx_sb[a][:, t * Ta * 8:(t + 1) * Ta * 8],
                        Ta * 128, Ta * 128, D, single_packet=False,
                        queue_num=qsel[0])
                    ps = ppool.tile([128, D], mybir.dt.float32, tag="ps",
                                    bufs=6, name="ps_seg")
                    for c in range(Ta):
                        mt = wpool.tile([128, 128], tdt, tag="mt", bufs=8,
                                        name="mt")
                        nc.vector.tensor_scalar(
                            out=mt[:], in0=iota[:],
                            scalar1=dst_sb[a][:, t * Ta + c:t * Ta + c + 1],
                            scalar2=val_sb[a][:, t * Ta + c:t * Ta + c + 1],
                            op0=OP.is_equal, op1=OP.mult)
                        nc.tensor.matmul(ps[:], mt[:], G[:, c, :],
                                         start=(c == 0), stop=(c == Ta - 1))
                    for key, w in drains.items():
                        if t in acc_written[key]:
                            tmp = wpool.tile([128, D], mybir.dt.float32,
                                             tag="tmp", bufs=6, name="tmp")
                            nc.scalar.activation(tmp[:], ps[:], AF.Copy,
                                                 scale=float(w))
                            nc.vector.tensor_tensor(
                                acc[key][:, t, :], acc[key][:, t, :], tmp[:],
                                op=OP.add)
                        else:
                            nc.scalar.activation(acc[key][:, t, :], ps[:],
                                                 AF.Copy, scale=float(w))
                    for key in drains:
                        acc_written[key].add(t)

            def write_state(key, s):
                for t in range(N_TILES):
                    st = wpool.tile([128, D], tdt, tag="stage", bufs=4,
                                    name="st_w")
                    nc.scalar.activation(st[:], acc[key][:, t, :], AF.Copy)
                    nc.sync.dma_start(slice_t[s][t * 128:(t + 1) * 128, :],
                                      st[:])
                nc.gpsimd.collective_compute(
                    "AllGather", OP.bypass, replica_groups=rg,
                    ins=[slice_t[s].opt()], outs=[full_t[s].opt()])

            wA, wB1, wC1 = weights["wA"], weights["wB1"], weights["wC1"]
            wB2, wC2, wC3 = weights["wB2"], weights["wC2"], weights["wC3"]

            for a in range(3):
                dr = {"A": wA[a], "B": wB1[a]}
                if a in wC1:
                    dr["C"] = wC1[a]
                seg_pass(full_t["s0"], a, dr)
            write_state("A", "s1")
            seg_pass(full_t["s0"], 3, {"B": wB1[3], "C": wC1[3]})

            for a in range(3):
                dr = {"B": wB2[a]}
                if a in wC2:
                    dr["C"] = wC2[a]
                seg_pass(full_t["s1"], a, dr)
            write_state("B", "s2")
            seg_pass(full_t["s1"], 3, {"C": wC2[3]})

            for a in range(2):
                seg_pass(full_t["s2"], a, {"C": wC3[a]})

            # ---- LayerNorm + exact GELU ----
            for t in range(N_TILES):
                y = accC[:, t, :]
                s1r = wpool.tile([128, 1], mybir.dt.float32, tag="ln1",
                                 bufs=2, name="s1r")
                nc.vector.tensor_reduce(s1r[:], y, axis=mybir.AxisListType.X,
                                        op=OP.add)
                mu = wpool.tile([128, 1], mybir.dt.float32, tag="ln2",
                                bufs=2, name="mu")
                nc.vector.tensor_scalar_mul(mu[:], s1r[:], 1.0 / D)
                yc = wpool.tile([128, D], mybir.dt.float32, tag="yc",
                                bufs=2, name="yc")
                nc.vector.tensor_scalar(out=yc[:], in0=y, scalar1=mu[:],
                                        scalar2=None, op0=OP.subtract)
                sq = wpool.tile([128, D], mybir.dt.float32, tag="sq",
                                bufs=2, name="sq")
                nc.scalar.activation(sq[:], yc[:], AF.Square)
                ss = wpool.tile([128, 1], mybir.dt.float32, tag="ln3",
                                bufs=2, name="ss")
                nc.vector.tensor_reduce(ss[:], sq[:],
                                        axis=mybir.AxisListType.X, op=OP.add)
                tv = wpool.tile([128, 1], mybir.dt.float32, tag="ln4",
                                bufs=2, name="tv")
                nc.vector.tensor_scalar(out=tv[:], in0=ss[:],
                                        scalar1=1.0 / D, scalar2=LN_EPS,
                                        op0=OP.mult, op1=OP.add)
                rinv = wpool.tile([128, 1], mybir.dt.float32, tag="ln5",
                                  bufs=2, name="rinv")
                nc.vector.reciprocal(rinv[:], tv[:])
                rstd = wpool.tile([128, 1], mybir.dt.float32, tag="ln6",
                                  bufs=2, name="rstd")
                nc.scalar.activation(rstd[:], rinv[:], AF.Sqrt)
                ot = wpool.tile([128, D], mybir.dt.float32, tag="ot",
                                bufs=3, name="ot")
                nc.scalar.activation(ot[:], yc[:], AF.Gelu, scale=rstd[:])
                nc.sync.dma_start(y_out[t * 128:(t + 1) * 128, :], ot[:])

    nc.compile()
    return nc


def kernel(x, rows, cols, vals, W, b, ws_seq_0, ws_seq_1, ws_res_0,
           ws_res_1):
    in_maps, T, padded_id = preprocess(x, rows, cols, vals)
    weights = make_weights(ws_seq_0, ws_seq_1, ws_res_0, ws_res_1)
    nc = build_program(T, weights, mode=MODE)

    bb = np.tile(np.asarray(b, dtype=np.float32)[None, :], (128, 1))
    iota_np = np.tile(np.arange(128, dtype=np.float32)[None, :], (128, 1))
    W_np = np.asarray(W, dtype=np.float32)
    for m in in_maps:
        m["W_in"] = W_np
        m["bb_in"] = bb
        m["iota_in"] = iota_np

    LAST_BUILD.clear()
    LAST_BUILD.update({"nc": nc, "in_maps": in_maps, "T": T})

    res = run_bass_kernel_spmd(nc, in_maps, core_ids=list(range(N_CORES)))
    y_all = np.concatenate(
        [res.results[c]["y_out"] for c in range(N_CORES)], axis=0)
    return y_all[padded_id].astype(np.float32)



# revision 9
# speedup vs baseline: 1.9407x; 1.9407x over previous
"""GNN message-passing (nn_Cell) kernel for Trainium2, 8-core SPMD Bass.

Strategy: destination-sharded SpMM. The 20000 nodes are split 2500/core
(padded to 2560 = 20 tiles x 128 rows). Per (core, adjacency), edges are
bucketed by destination tile and padded to a uniform T chunks of 128 edges
per tile. Each of the 10 (source-state, adjacency) segment-sums runs as:

  dma_gather   : pull the 256-float source rows for 128*T edges from the
                 replicated state table in HBM (one edge per partition)
  DVE          : build one-hot MT[e, d] = (iota[d] == dst_e) * val_e per
                 128-edge chunk (iota kept in PSUM so the op runs in 1x
                 mode and never locks the SBUF port SWDGE needs)
  TensorE      : psum[128 dst, 256] += MT.T @ G_chunk   (PSUM accumulate)

Weighted PSUM drains (ACT scale + DVE add) feed three persistent SBUF
accumulators (s1 / s2 / out). An AllGather shares each newly computed state
across the 8 cores between rounds; the adjacency-3 pass of each round is
scheduled after the AllGather so it hides the collective latency. Final
LayerNorm + exact GELU run fused on DVE/ACT.

State tables are stored as float32r (TF32-like), giving ~1e-4 relative
error while the one-hot matmuls run at full PE rate.
"""
import sys

for _p in ("/opt/trn_rl_repo",):
    if _p not in sys.path:
        sys.path.insert(0, _p)

import numpy as np
import concourse.bacc as bacc
import concourse.tile as tile
from concourse import mybir
from concourse.bass_utils import run_bass_kernel_spmd

N_CORES = 8
N_NODES = 20000
N_ADJ = 4
D = 256
ROWS_PER = N_NODES // N_CORES          # 2500
N_TILES = (ROWS_PER + 127) // 128      # 20
PAD_ROWS = N_TILES * 128               # 2560
PADN = PAD_ROWS * N_CORES              # 20480
LN_EPS = 1e-5

MODE = "bf16"                          # "f32" | "f32r" | "bf16"
IOTA = "sbuf_16"                       # "psum_f32" | "sbuf_16"
DT_MAP = {
    "f32": mybir.dt.float32,
    "f32r": mybir.dt.float32r,
    "bf16": mybir.dt.bfloat16,
}

LAST_BUILD = {}


def balance_rows(rows):
    """Greedy assignment of nodes to (core, tile, slot) balancing the four
    per-adjacency in-degree loads across all 160 destination tiles, so every
    tile needs the same (minimal) number of 128-edge chunks."""
    deg = np.stack([np.bincount(rows[a], minlength=N_NODES)
                    for a in range(N_ADJ)], 1)
    order = np.argsort(-deg.sum(1))
    NB = N_CORES * N_TILES
    load = np.zeros((NB, N_ADJ), dtype=np.int64)
    bcount = np.zeros(NB, dtype=np.int32)
    ccount = np.zeros(N_CORES, dtype=np.int32)
    core_of = np.zeros(N_NODES, np.int32)
    tile_of = np.zeros(N_NODES, np.int32)
    slot_of = np.zeros(N_NODES, np.int32)
    bucket_core = np.arange(NB) // N_TILES
    BIG = 1 << 60
    for n in order:
        dn = deg[n]
        score = (load + dn).max(1)
        score[(bcount >= 128) | (ccount[bucket_core] >= ROWS_PER)] = BIG
        b = int(np.argmin(score))
        load[b] += dn
        core_of[n] = bucket_core[b]
        tile_of[n] = b % N_TILES
        slot_of[n] = bcount[b]
        bcount[b] += 1
        ccount[bucket_core[b]] += 1
    return core_of, tile_of, slot_of


def preprocess(x, rows, cols, vals):
    """Host-side edge partitioning. Returns per-core input maps, T, and the
    node -> padded-row permutation."""
    rows = np.asarray(rows).astype(np.int64)
    cols = np.asarray(cols).astype(np.int64)
    vals = np.asarray(vals).astype(np.float32)
    x = np.asarray(x).astype(np.float32)

    core_of, tile_of, slot_of = balance_rows(rows)
    padded_id = core_of * PAD_ROWS + tile_of * 128 + slot_of

    core = core_of[rows]
    tl = tile_of[rows]
    dst = slot_of[rows].astype(np.float32)
    gcol = padded_id[cols]

    T = []
    for a in range(N_ADJ):
        key = core[a] * N_TILES + tl[a]
        cnt = np.bincount(key, minlength=N_CORES * N_TILES)
        T.append(int(np.ceil(cnt.max() / 128)))

    idx_np = [np.zeros((N_CORES, 128, N_TILES * T[a] * 8), dtype=np.int16)
              for a in range(N_ADJ)]
    dst_np = [np.zeros((N_CORES, 128, N_TILES * T[a]), dtype=np.float32)
              for a in range(N_ADJ)]
    val_np = [np.zeros((N_CORES, 128, N_TILES * T[a]), dtype=np.float32)
              for a in range(N_ADJ)]
    for a in range(N_ADJ):
        Ta = T[a]
        key = core[a] * N_TILES + tl[a]
        order = np.argsort(key, kind='stable')
        key_s = key[order]
        gcol_s = gcol[a][order].astype(np.int16)
        dst_s = dst[a][order]
        val_s = vals[a][order]
        bound = np.searchsorted(key_s, np.arange(N_CORES * N_TILES))
        j = np.arange(len(key_s)) - bound[key_s]
        c_arr = key_s // N_TILES
        t_arr = key_s % N_TILES
        idx_np[a][c_arr, j % 16, t_arr * Ta * 8 + j // 16] = gcol_s
        dst_np[a][c_arr, j % 128, t_arr * Ta + j // 128] = dst_s
        val_np[a][c_arr, j % 128, t_arr * Ta + j // 128] = val_s
        for r in range(1, 8):
            idx_np[a][:, 16 * r:16 * (r + 1), :] = idx_np[a][:, :16, :]

    xT = np.zeros((N_CORES, D, PAD_ROWS), dtype=np.float32)
    xT[core_of, :, tile_of * 128 + slot_of] = x

    in_maps = []
    for c in range(N_CORES):
        m = {"xT_in": xT[c]}
        for a in range(N_ADJ):
            m[f"idx{a}_in"] = idx_np[a][c]
            m[f"dst{a}_in"] = dst_np[a][c]
            m[f"val{a}_in"] = val_np[a][c]
        in_maps.append(m)
    return in_maps, T, padded_id


def make_weights(ws_seq_0, ws_seq_1, ws_res_0, ws_res_1):
    ws_seq_0 = np.asarray(ws_seq_0, dtype=np.float64)
    ws_seq_1 = np.asarray(ws_seq_1, dtype=np.float64)
    ws_res_0 = np.asarray(ws_res_0, dtype=np.float64)
    ws_res_1 = np.asarray(ws_res_1, dtype=np.float64)
    return {
        "wA": {a: ws_seq_0[0][a] / 3 for a in range(3)},
        "wB1": {a: ws_res_0[0][a] / 4 for a in range(4)},
        "wC1": {0: ws_res_1[0][0] / 3, 1: ws_res_1[0][1] / 3,
                3: ws_res_1[0][2] / 3},
        "wB2": {a: ws_seq_0[1][a] / 3 for a in range(3)},
        "wC2": {0: ws_res_1[1][0] / 3, 1: ws_res_1[1][1] / 3,
                3: ws_res_1[1][2] / 3},
        "wC3": {0: ws_seq_1[0] / 2, 1: ws_seq_1[1] / 2},
    }


def build_program(T, weights, mode=MODE, sim_single_core=False,
                  fake_collective=False):
    tdt = DT_MAP[mode]
    ndev = 1 if sim_single_core else N_CORES
    fake_collective = fake_collective or sim_single_core
    nc = bacc.Bacc("TRN2", target_bir_lowering=False, debug=False,
                   num_devices=ndev, num_swdge_queues=2)

    xT_in = nc.dram_tensor("xT_in", [D, PAD_ROWS], mybir.dt.float32,
                           kind="ExternalInput")
    W_in = nc.dram_tensor("W_in", [D, D], mybir.dt.float32,
                          kind="ExternalInput")
    bb_in = nc.dram_tensor("bb_in", [128, D], mybir.dt.float32,
                           kind="ExternalInput")
    iota_in = nc.dram_tensor("iota_in", [128, 128], mybir.dt.float32,
                             kind="ExternalInput")
    idx_in, dstv_in, valv_in = [], [], []
    for a in range(N_ADJ):
        idx_in.append(nc.dram_tensor(f"idx{a}_in", [128, N_TILES * T[a] * 8],
                                     mybir.dt.int16, kind="ExternalInput"))
        dstv_in.append(nc.dram_tensor(f"dst{a}_in", [128, N_TILES * T[a]],
                                      mybir.dt.float32, kind="ExternalInput"))
        valv_in.append(nc.dram_tensor(f"val{a}_in", [128, N_TILES * T[a]],
                                      mybir.dt.float32, kind="ExternalInput"))
    y_out = nc.dram_tensor("y_out", [PAD_ROWS, D], mybir.dt.float32,
                           kind="ExternalOutput")

    rg = [list(range(N_CORES))]
    AF = mybir.ActivationFunctionType
    OP = mybir.AluOpType

    with tile.TileContext(nc) as tc:
        with tc.tile_pool(name="const", bufs=1) as cpool, \
             tc.tile_pool(name="work", bufs=1) as wpool, \
             tc.tile_pool(name="psum", bufs=4, space="PSUM") as ppool, \
             tc.tile_pool(name="dram", bufs=1, space="DRAM") as dpool:

            W_sb = cpool.tile([128, 2, D], mybir.dt.float32, name="W_sb")
            nc.sync.dma_start(W_sb[:],
                              W_in.ap().rearrange("(k p) d -> p k d", p=128))
            bb = cpool.tile([128, D], mybir.dt.float32, name="bb")
            nc.sync.dma_start(bb[:], bb_in[:])
            iota_sb = cpool.tile([128, 128], mybir.dt.float32, name="iota_sb")
            nc.sync.dma_start(iota_sb[:], iota_in[:])
            if IOTA == "psum_f32":
                # iota in PSUM: forces the MT-build tensor_scalar into 1x
                # mode so DVE never takes the SBUF port SWDGE (dma_gather
                # descriptor generation) depends on.
                iota = ppool.tile([128, 128], mybir.dt.float32,
                                  tag="iota_ps", bufs=1, name="iota_ps")
                nc.vector.tensor_copy(iota[:], iota_sb[:])
            else:
                # 16-bit iota in SBUF: all non-scalar MT-build operands are
                # 2-byte SBUF tiles, unlocking the DVE 2x/4x perf modes.
                iota = cpool.tile([128, 128], mybir.dt.bfloat16,
                                  name="iota_16")
                nc.vector.tensor_copy(iota[:], iota_sb[:])

            idx_sb, dst_sb, val_sb = [], [], []
            for a in range(N_ADJ):
                t_ = cpool.tile([128, N_TILES * T[a] * 8], mybir.dt.int16,
                                name=f"idx_sb{a}")
                nc.sync.dma_start(t_[:], idx_in[a][:])
                idx_sb.append(t_)
                t_ = cpool.tile([128, N_TILES * T[a]], mybir.dt.float32,
                                name=f"dst_sb{a}")
                nc.sync.dma_start(t_[:], dstv_in[a][:])
                dst_sb.append(t_)
                t_ = cpool.tile([128, N_TILES * T[a]], mybir.dt.float32,
                                name=f"val_sb{a}")
                nc.sync.dma_start(t_[:], valv_in[a][:])
                val_sb.append(t_)

            accA = cpool.tile([128, N_TILES, D], mybir.dt.float32, name="accA")
            accB = cpool.tile([128, N_TILES, D], mybir.dt.float32, name="accB")
            accC = cpool.tile([128, N_TILES, D], mybir.dt.float32, name="accC")
            acc = {"A": accA, "B": accB, "C": accC}
            acc_written = {k: set() for k in acc}

            slice_t, full_t = {}, {}
            for s in ("s0", "s1", "s2"):
                slice_t[s] = dpool.tile([PAD_ROWS, D], tdt, name=f"{s}_slice")
                aspace = {} if sim_single_core else {"addr_space": "Shared"}
                full_t[s] = dpool.tile([PADN, D], tdt, name=f"{s}_full",
                                       **aspace)

            # ---- affine: s0 = x @ W + b (own rows only) ----
            def all_gather(s):
                if fake_collective:
                    nc.sync.dma_start(full_t[s][0:PAD_ROWS, :],
                                      slice_t[s][:])
                else:
                    nc.gpsimd.collective_compute(
                        "AllGather", OP.bypass, replica_groups=rg,
                        ins=[slice_t[s].opt()], outs=[full_t[s].opt()])

            for t in range(N_TILES):
                xk = wpool.tile([128, 2, 128], mybir.dt.float32, tag="xk",
                                bufs=3, name="xk")
                nc.sync.dma_start(
                    xk[:], xT_in.ap().rearrange("(k p) r -> p k r", p=128)
                    [:, :, t * 128:(t + 1) * 128])
                ps = ppool.tile([128, D], mybir.dt.float32, tag="ps", bufs=6,
                                name="ps_aff")
                for k in range(2):
                    nc.tensor.matmul(ps[:], xk[:, k, :], W_sb[:, k, :],
                                     start=(k == 0), stop=(k == 1))
                st = wpool.tile([128, D], tdt, tag="stage", bufs=4,
                                name="st_aff")
                nc.vector.tensor_tensor(st[:], ps[:], bb[:], op=OP.add)
                nc.sync.dma_start(slice_t["s0"][t * 128:(t + 1) * 128, :],
                                  st[:])
            all_gather("s0")

            qsel = [0]

            def seg_pass(src_full, a, drains):
                Ta = T[a]
                for t in range(N_TILES):
                    G = wpool.tile([128, max(T), D], tdt, tag="G", bufs=4,
                                   name="G")[:, :Ta, :]
                    qsel[0] ^= 1
                    nc.gpsimd.dma_gather(
                        G, src_full[:],
                        idx_sb[a][:, t * Ta * 8:(t + 1) * Ta * 8],
                        Ta * 128, Ta * 128, D, single_packet=False,
                        queue_num=qsel[0])
                    ps = ppool.tile([128, D], mybir.dt.float32, tag="ps",
                                    bufs=6, name="ps_seg")
                    for c in range(Ta):
                        mt = wpool.tile([128, 128], tdt, tag="mt", bufs=8,
                                        name="mt")
                        nc.vector.tensor_scalar(
                            out=mt[:], in0=iota[:],
                            scalar1=dst_sb[a][:, t * Ta + c:t * Ta + c + 1],
                            scalar2=val_sb[a][:, t * Ta + c:t * Ta + c + 1],
                            op0=OP.is_equal, op1=OP.mult)
                        nc.tensor.matmul(ps[:], mt[:], G[:, c, :],
                                         start=(c == 0), stop=(c == Ta - 1))
                    for key, w in drains.items():
                        if t in acc_written[key]:
                            tmp = wpool.tile([128, D], mybir.dt.float32,
                                             tag="tmp", bufs=6, name="tmp")
                            nc.scalar.activation(tmp[:], ps[:], AF.Copy,
                                                 scale=float(w))
                            nc.vector.tensor_tensor(
                                acc[key][:, t, :], acc[key][:, t, :], tmp[:],
                                op=OP.add)
                        else:
                            nc.scalar.activation(acc[key][:, t, :], ps[:],
                                                 AF.Copy, scale=float(w))
                    for key in drains:
                        acc_written[key].add(t)

            def write_state(key, s):
                for t in range(N_TILES):
                    st = wpool.tile([128, D], tdt, tag="stage", bufs=4,
                                    name="st_w")
                    nc.scalar.activation(st[:], acc[key][:, t, :], AF.Copy)
                    nc.sync.dma_start(slice_t[s][t * 128:(t + 1) * 128, :],
                                      st[:])
                all_gather(s)

            wA, wB1, wC1 = weights["wA"], weights["wB1"], weights["wC1"]
            wB2, wC2, wC3 = weights["wB2"], weights["wC2"], weights["wC3"]

            for a in range(3):
                dr = {"A": wA[a], "B": wB1[a]}
                if a in wC1:
                    dr["C"] = wC1[a]
                seg_pass(full_t["s0"], a, dr)
            write_state("A", "s1")
            seg_pass(full_t["s0"], 3, {"B": wB1[3], "C": wC1[3]})

            for a in range(3):
                dr = {"B": wB2[a]}
                if a in wC2:
                    dr["C"] = wC2[a]
                seg_pass(full_t["s1"], a, dr)
            write_state("B", "s2")
            seg_pass(full_t["s1"], 3, {"C": wC2[3]})

            for a in range(2):
                seg_pass(full_t["s2"], a, {"C": wC3[a]})

            # ---- LayerNorm + exact GELU ----
            for t in range(N_TILES):
                y = accC[:, t, :]
                s1r = wpool.tile([128, 1], mybir.dt.float32, tag="ln1",
                                 bufs=2, name="s1r")
                nc.vector.tensor_reduce(s1r[:], y, axis=mybir.AxisListType.X,
                                        op=OP.add)
                mu = wpool.tile([128, 1], mybir.dt.float32, tag="ln2",
                                bufs=2, name="mu")
                nc.vector.tensor_scalar_mul(mu[:], s1r[:], 1.0 / D)
                yc = wpool.tile([128, D], mybir.dt.float32, tag="yc",
                                bufs=2, name="yc")
                nc.vector.tensor_scalar(out=yc[:], in0=y, scalar1=mu[:],
                                        scalar2=None, op0=OP.subtract)
                sq = wpool.tile([128, D], mybir.dt.float32, tag="sq",
                                bufs=2, name="sq")
                nc.scalar.activation(sq[:], yc[:], AF.Square)
                ss = wpool.tile([128, 1], mybir.dt.float32, tag="ln3",
                                bufs=2, name="ss")
                nc.vector.tensor_reduce(ss[:], sq[:],
                                        axis=mybir.AxisListType.X, op=OP.add)
                tv = wpool.tile([128, 1], mybir.dt.float32, tag="ln4",
                                bufs=2, name="tv")
                nc.vector.tensor_scalar(out=tv[:], in0=ss[:],
                                        scalar1=1.0 / D, scalar2=LN_EPS,
                                        op0=OP.mult, op1=OP.add)
                rinv = wpool.tile([128, 1], mybir.dt.float32, tag="ln5",
                                  bufs=2, name="rinv")
                nc.vector.reciprocal(rinv[:], tv[:])
                rstd = wpool.tile([128, 1], mybir.dt.float32, tag="ln6",
                                  bufs=2, name="rstd")
                nc.scalar.activation(rstd[:], rinv[:], AF.Sqrt)
                ot = wpool.tile([128, D], mybir.dt.float32, tag="ot",
                                bufs=3, name="ot")
                nc.scalar.activation(ot[:], yc[:], AF.Gelu, scale=rstd[:])
                nc.sync.dma_start(y_out[t * 128:(t + 1) * 128, :], ot[:])

    nc.compile()
    return nc


def kernel(x, rows, cols, vals, W, b, ws_seq_0, ws_seq_1, ws_res_0,
           ws_res_1):
    in_maps, T, padded_id = preprocess(x, rows, cols, vals)
    weights = make_weights(ws_seq_0, ws_seq_1, ws_res_0, ws_res_1)
    nc = build_program(T, weights, mode=MODE)

    bb = np.tile(np.asarray(b, dtype=np.float32)[None, :], (128, 1))
    iota_np = np.tile(np.arange(128, dtype=np.float32)[None, :], (128, 1))
    W_np = np.asarray(W, dtype=np.float32)
    for m in in_maps:
        m["W_in"] = W_np
        m["bb_in"] = bb
        m["iota_in"] = iota_np

    LAST_BUILD.clear()
    LAST_BUILD.update({"nc": nc, "in_maps": in_maps, "T": T})

    res = run_bass_kernel_spmd(nc, in_maps, core_ids=list(range(N_CORES)))
    y_all = np.concatenate(
        [res.results[c]["y_out"] for c in range(N_CORES)], axis=0)
    return y_all[padded_id].astype(np.float32)



# revision 24
# speedup vs baseline: 4.4842x; 2.3106x over previous
"""GNN message-passing (nn_Cell) kernel for Trainium2, 8-core SPMD Bass.

Strategy: destination-sharded SpMM. The 20000 nodes are split 2500/core
(padded to 2560 = 20 tiles x 128 rows). Per (core, adjacency), edges are
bucketed by destination tile and padded to a uniform T chunks of 128 edges
per tile. Each of the 10 (source-state, adjacency) segment-sums runs as:

  dma_gather   : pull the 256-float source rows for 128*T edges from the
                 replicated state table in HBM (one edge per partition)
  DVE          : build one-hot MT[e, d] = (iota[d] == dst_e) * val_e per
                 128-edge chunk (iota kept in PSUM so the op runs in 1x
                 mode and never locks the SBUF port SWDGE needs)
  TensorE      : psum[128 dst, 256] += MT.T @ G_chunk   (PSUM accumulate)

Weighted PSUM drains (ACT scale + DVE add) feed three persistent SBUF
accumulators (s1 / s2 / out). An AllGather shares each newly computed state
across the 8 cores between rounds; the adjacency-3 pass of each round is
scheduled after the AllGather so it hides the collective latency. Final
LayerNorm + exact GELU run fused on DVE/ACT.

State tables are stored as float32r (TF32-like), giving ~1e-4 relative
error while the one-hot matmuls run at full PE rate.
"""
import sys

for _p in ("/opt/trn_rl_repo",):
    if _p not in sys.path:
        sys.path.insert(0, _p)

import numpy as np
import concourse.bacc as bacc
import concourse.tile as tile
from concourse import mybir
from concourse.bass_utils import run_bass_kernel_spmd

N_CORES = 8
N_NODES = 20000
N_ADJ = 4
D = 256
ROWS_PER = N_NODES // N_CORES          # 2500
N_TILES = (ROWS_PER + 127) // 128      # 20
PAD_ROWS = N_TILES * 128               # 2560
PADN = PAD_ROWS * N_CORES              # 20480
LN_EPS = 1e-5

MODE = "bf16"                          # "f32" | "f32r" | "bf16"
IOTA = "psum_f32"                      # "psum_f32" | "sbuf_16"
N_QUEUES = 4                           # SWDGE queues (max 4)
DT_MAP = {
    "f32": mybir.dt.float32,
    "f32r": mybir.dt.float32r,
    "bf16": mybir.dt.bfloat16,
}

LAST_BUILD = {}


def balance_rows(rows):
    """Greedy assignment of nodes to (core, tile, slot) balancing the four
    per-adjacency in-degree loads across all 160 destination tiles, so every
    tile needs the same (minimal) number of 128-edge chunks."""
    deg = np.stack([np.bincount(rows[a], minlength=N_NODES)
                    for a in range(N_ADJ)], 1)
    order = np.argsort(-deg.sum(1))
    NB = N_CORES * N_TILES
    load = np.zeros((NB, N_ADJ), dtype=np.int64)
    bcount = np.zeros(NB, dtype=np.int32)
    ccount = np.zeros(N_CORES, dtype=np.int32)
    core_of = np.zeros(N_NODES, np.int32)
    tile_of = np.zeros(N_NODES, np.int32)
    slot_of = np.zeros(N_NODES, np.int32)
    bucket_core = np.arange(NB) // N_TILES
    BIG = 1 << 60
    for n in order:
        dn = deg[n]
        score = (load + dn).max(1)
        score[(bcount >= 128) | (ccount[bucket_core] >= ROWS_PER)] = BIG
        b = int(np.argmin(score))
        load[b] += dn
        core_of[n] = bucket_core[b]
        tile_of[n] = b % N_TILES
        slot_of[n] = bcount[b]
        bcount[b] += 1
        ccount[bucket_core[b]] += 1
    return core_of, tile_of, slot_of


def preprocess(x, rows, cols, vals):
    """Host-side edge partitioning. Returns per-core input maps, T, and the
    node -> padded-row permutation."""
    rows = np.asarray(rows).astype(np.int64)
    cols = np.asarray(cols).astype(np.int64)
    vals = np.asarray(vals).astype(np.float32)
    x = np.asarray(x).astype(np.float32)

    core_of, tile_of, slot_of = balance_rows(rows)
    padded_id = core_of * PAD_ROWS + tile_of * 128 + slot_of

    core = core_of[rows]
    tl = tile_of[rows]
    dst = slot_of[rows].astype(np.float32)
    gcol = padded_id[cols]

    T = []
    for a in range(N_ADJ):
        key = core[a] * N_TILES + tl[a]
        cnt = np.bincount(key, minlength=N_CORES * N_TILES)
        T.append(int(np.ceil(cnt.max() / 128)))

    import ml_dtypes
    bf16 = ml_dtypes.bfloat16
    idx_np = [np.zeros((N_CORES, 128, N_TILES * T[a] * 8), dtype=np.int16)
              for a in range(N_ADJ)]
    # pre-built one-hot matmul tiles: mtx[core, e, (t*Ta+c)*128 + d] =
    # val * (dst == d) for the edge at (tile t, chunk c, slot e)
    mtx_np = [np.zeros((N_CORES, 128, N_TILES * T[a] * 128), dtype=bf16)
              for a in range(N_ADJ)]
    for a in range(N_ADJ):
        Ta = T[a]
        key = core[a] * N_TILES + tl[a]
        # secondary sort by source row: consecutive gather descriptors hit
        # ascending HBM addresses (row-buffer locality)
        order = np.lexsort((gcol[a], key))
        key_s = key[order]
        gcol_s = gcol[a][order].astype(np.int16)
        dst_s = dst[a][order].astype(np.int64)
        val_s = vals[a][order]
        bound = np.searchsorted(key_s, np.arange(N_CORES * N_TILES))
        j = np.arange(len(key_s)) - bound[key_s]
        c_arr = key_s // N_TILES
        t_arr = key_s % N_TILES
        idx_np[a][c_arr, j % 16, t_arr * Ta * 8 + j // 16] = gcol_s
        mtx_np[a][c_arr, j % 128,
                  (t_arr * Ta + j // 128) * 128 + dst_s] = val_s
        for r in range(1, 8):
            idx_np[a][:, 16 * r:16 * (r + 1), :] = idx_np[a][:, :16, :]

    xT = np.zeros((N_CORES, D, PAD_ROWS), dtype=np.float32)
    xT[core_of, :, tile_of * 128 + slot_of] = x

    in_maps = []
    for c in range(N_CORES):
        m = {"xT_in": xT[c]}
        for a in range(N_ADJ):
            m[f"idx{a}_in"] = idx_np[a][c]
            m[f"mtx{a}_in"] = mtx_np[a][c]
        in_maps.append(m)
    return in_maps, T, padded_id


def make_weights(ws_seq_0, ws_seq_1, ws_res_0, ws_res_1):
    ws_seq_0 = np.asarray(ws_seq_0, dtype=np.float64)
    ws_seq_1 = np.asarray(ws_seq_1, dtype=np.float64)
    ws_res_0 = np.asarray(ws_res_0, dtype=np.float64)
    ws_res_1 = np.asarray(ws_res_1, dtype=np.float64)
    return {
        "wA": {a: ws_seq_0[0][a] / 3 for a in range(3)},
        "wB1": {a: ws_res_0[0][a] / 4 for a in range(4)},
        "wC1": {0: ws_res_1[0][0] / 3, 1: ws_res_1[0][1] / 3,
                3: ws_res_1[0][2] / 3},
        "wB2": {a: ws_seq_0[1][a] / 3 for a in range(3)},
        "wC2": {0: ws_res_1[1][0] / 3, 1: ws_res_1[1][1] / 3,
                3: ws_res_1[1][2] / 3},
        "wC3": {0: ws_seq_1[0] / 2, 1: ws_seq_1[1] / 2},
    }


def build_program(T, weights, mode=MODE, sim_single_core=False,
                  fake_collective=False, skip_mt=False, skip_gather=False):
    tdt = DT_MAP[mode]
    ndev = 1 if sim_single_core else N_CORES
    fake_collective = fake_collective or sim_single_core
    nc = bacc.Bacc("TRN2", target_bir_lowering=False, debug=False,
                   num_devices=ndev, num_swdge_queues=N_QUEUES)

    xT_in = nc.dram_tensor("xT_in", [D, PAD_ROWS], mybir.dt.float32,
                           kind="ExternalInput")
    W_in = nc.dram_tensor("W_in", [D, D], mybir.dt.float32,
                          kind="ExternalInput")
    bb_in = nc.dram_tensor("bb_in", [128, D], mybir.dt.float32,
                           kind="ExternalInput")
    iota_in = nc.dram_tensor("iota_in", [128, 128], mybir.dt.float32,
                             kind="ExternalInput")
    idx_in, mtx_in = [], []
    for a in range(N_ADJ):
        idx_in.append(nc.dram_tensor(f"idx{a}_in", [128, N_TILES * T[a] * 8],
                                     mybir.dt.int16, kind="ExternalInput"))
        mtx_in.append(nc.dram_tensor(f"mtx{a}_in",
                                     [128, N_TILES * T[a] * 128],
                                     mybir.dt.bfloat16,
                                     kind="ExternalInput"))
    y_out = nc.dram_tensor("y_out", [PAD_ROWS, D], mybir.dt.float32,
                           kind="ExternalOutput")

    rg = [list(range(N_CORES))]
    AF = mybir.ActivationFunctionType
    OP = mybir.AluOpType

    with tile.TileContext(nc) as tc:
        with tc.tile_pool(name="const", bufs=1) as cpool, \
             tc.tile_pool(name="work", bufs=1) as wpool, \
             tc.tile_pool(name="psum", bufs=4, space="PSUM") as ppool, \
             tc.tile_pool(name="dram", bufs=1, space="DRAM") as dpool:

            W_sb = cpool.tile([128, 2, D], mybir.dt.float32, name="W_sb")
            nc.sync.dma_start(W_sb[:],
                              W_in.ap().rearrange("(k p) d -> p k d", p=128))
            bb = cpool.tile([128, D], mybir.dt.float32, name="bb")
            nc.sync.dma_start(bb[:], bb_in[:])
            iota_sb = cpool.tile([128, 128], mybir.dt.float32, name="iota_sb")
            nc.sync.dma_start(iota_sb[:], iota_in[:])

            idx_sb = []
            for a in range(N_ADJ):
                t_ = cpool.tile([128, N_TILES * T[a] * 8], mybir.dt.int16,
                                name=f"idx_sb{a}")
                nc.sync.dma_start(t_[:], idx_in[a][:])
                idx_sb.append(t_)

            mt_const = None
            if skip_mt:
                mt_const = cpool.tile([128, 128], tdt, name="mt_const")
                nc.vector.tensor_scalar(
                    out=mt_const[:], in0=iota_sb[:], scalar1=1.0,
                    scalar2=None, op0=OP.mult)
                mt_const = mt_const[:]
            g_const = None
            if skip_gather:
                g_const = cpool.tile([128, max(T), D], tdt, name="g_const")
                for gk in range(max(T)):
                    nc.vector.tensor_scalar(
                        out=g_const[:, gk, 0:128], in0=iota_sb[:],
                        scalar1=1.0, scalar2=None, op0=OP.mult)
                    nc.vector.tensor_scalar(
                        out=g_const[:, gk, 128:256], in0=iota_sb[:],
                        scalar1=1.0, scalar2=None, op0=OP.mult)

            accA = cpool.tile([128, N_TILES, D], mybir.dt.float32, name="accA")
            accB = cpool.tile([128, N_TILES, D], mybir.dt.float32, name="accB")
            accC = cpool.tile([128, N_TILES, D], mybir.dt.float32, name="accC")
            acc = {"A": accA, "B": accB, "C": accC}
            acc_written = {k: set() for k in acc}

            slice_t, full_t = {}, {}
            for s in ("s0", "s1", "s2"):
                slice_t[s] = dpool.tile([PAD_ROWS, D], tdt, name=f"{s}_slice")
                aspace = {} if sim_single_core else {"addr_space": "Shared"}
                full_t[s] = dpool.tile([PADN, D], tdt, name=f"{s}_full",
                                       **aspace)

            # ---- affine: s0 = x @ W + b (own rows only) ----
            def all_gather(s):
                if fake_collective:
                    nc.sync.dma_start(full_t[s][0:PAD_ROWS, :],
                                      slice_t[s][:])
                else:
                    nc.gpsimd.collective_compute(
                        "AllGather", OP.bypass, replica_groups=rg,
                        ins=[slice_t[s].opt()], outs=[full_t[s].opt()])

            for t in range(N_TILES):
                xk = wpool.tile([128, 2, 128], mybir.dt.float32, tag="xk",
                                bufs=3, name="xk")
                nc.sync.dma_start(
                    xk[:], xT_in.ap().rearrange("(k p) r -> p k r", p=128)
                    [:, :, t * 128:(t + 1) * 128])
                ps = ppool.tile([128, D], mybir.dt.float32, tag="ps", bufs=6,
                                name="ps_aff")
                for k in range(2):
                    nc.tensor.matmul(ps[:], xk[:, k, :], W_sb[:, k, :],
                                     start=(k == 0), stop=(k == 1))
                st = wpool.tile([128, D], tdt, tag="stage", bufs=4,
                                name="st_aff")
                nc.vector.tensor_tensor(st[:], ps[:], bb[:], op=OP.add)
                nc.sync.dma_start(slice_t["s0"][t * 128:(t + 1) * 128, :],
                                  st[:])
            all_gather("s0")

            qsel = [0]

            def seg_pass(src_full, a, drains):
                Ta = T[a]

                def make_drain(ps, t):
                    # snapshot write-state NOW; executed one tile later
                    todo = [(key, w, t in acc_written[key])
                            for key, w in drains.items()]
                    for key in drains:
                        acc_written[key].add(t)

                    def emit():
                        for key, w, written in todo:
                            if written:
                                tmp = wpool.tile([128, D], mybir.dt.float32,
                                                 tag="tmp", bufs=6,
                                                 name="tmp")
                                nc.scalar.activation(tmp[:], ps[:], AF.Copy,
                                                     scale=float(w))
                                nc.vector.tensor_tensor(
                                    acc[key][:, t, :], acc[key][:, t, :],
                                    tmp[:], op=OP.add)
                            else:
                                nc.scalar.activation(acc[key][:, t, :],
                                                     ps[:], AF.Copy,
                                                     scale=float(w))
                    return emit

                pending = None
                for t in range(N_TILES):
                    if skip_gather:
                        G = g_const[:, :Ta, :]
                    else:
                        G = wpool.tile([128, max(T), D], tdt, tag="G",
                                       bufs=6, name="G")[:, :Ta, :]
                        qsel[0] = (qsel[0] + 1) % N_QUEUES
                        nc.gpsimd.dma_gather(
                            G, src_full[:],
                            idx_sb[a][:, t * Ta * 8:(t + 1) * Ta * 8],
                            Ta * 128, Ta * 128, D, single_packet=False,
                            queue_num=qsel[0])
                    if skip_mt:
                        mts = None
                    else:
                        # host-prebuilt one-hot tiles streamed from HBM:
                        # no per-chunk producer instructions on any engine
                        mts = wpool.tile([128, max(T), 128], tdt, tag="mts",
                                         bufs=6, name="mts")
                        nc.sync.dma_start(
                            mts[:, :Ta, :],
                            mtx_in[a][:, t * Ta * 128:(t + 1) * Ta * 128])
                    ps = ppool.tile([128, D], mybir.dt.float32, tag="ps",
                                    bufs=6, name="ps_seg")
                    for c in range(Ta):
                        mt = mt_const if skip_mt else mts[:, c, :]
                        nc.tensor.matmul(ps[:], mt, G[:, c, :],
                                         start=(c == 0), stop=(c == Ta - 1))
                    # drains of tile t-1 issue after tile t's matmuls so the
                    # DVE add never gates the next tile's dispatch
                    if pending is not None:
                        pending()
                    pending = make_drain(ps, t)
                pending()

            def write_state(key, s):
                for t in range(N_TILES):
                    st = wpool.tile([128, D], tdt, tag="stage", bufs=4,
                                    name="st_w")
                    nc.scalar.activation(st[:], acc[key][:, t, :], AF.Copy)
                    nc.sync.dma_start(slice_t[s][t * 128:(t + 1) * 128, :],
                                      st[:])
                all_gather(s)

            wA, wB1, wC1 = weights["wA"], weights["wB1"], weights["wC1"]
            wB2, wC2, wC3 = weights["wB2"], weights["wC2"], weights["wC3"]

            for a in range(3):
                dr = {"A": wA[a], "B": wB1[a]}
                if a in wC1:
                    dr["C"] = wC1[a]
                seg_pass(full_t["s0"], a, dr)
            write_state("A", "s1")
            seg_pass(full_t["s0"], 3, {"B": wB1[3], "C": wC1[3]})

            for a in range(3):
                dr = {"B": wB2[a]}
                if a in wC2:
                    dr["C"] = wC2[a]
                seg_pass(full_t["s1"], a, dr)
            write_state("B", "s2")
            seg_pass(full_t["s1"], 3, {"C": wC2[3]})

            for a in range(2):
                seg_pass(full_t["s2"], a, {"C": wC3[a]})

            # ---- LayerNorm + exact GELU ----
            for t in range(N_TILES):
                y = accC[:, t, :]
                s1r = wpool.tile([128, 1], mybir.dt.float32, tag="ln1",
                                 bufs=2, name="s1r")
                nc.vector.tensor_reduce(s1r[:], y, axis=mybir.AxisListType.X,
                                        op=OP.add)
                mu = wpool.tile([128, 1], mybir.dt.float32, tag="ln2",
                                bufs=2, name="mu")
                nc.vector.tensor_scalar_mul(mu[:], s1r[:], 1.0 / D)
                yc = wpool.tile([128, D], mybir.dt.float32, tag="yc",
                                bufs=2, name="yc")
                nc.vector.tensor_scalar(out=yc[:], in0=y, scalar1=mu[:],
                                        scalar2=None, op0=OP.subtract)
                sq = wpool.tile([128, D], mybir.dt.float32, tag="sq",
                                bufs=2, name="sq")
                nc.scalar.activation(sq[:], yc[:], AF.Square)
                ss = wpool.tile([128, 1], mybir.dt.float32, tag="ln3",
                                bufs=2, name="ss")
                nc.vector.tensor_reduce(ss[:], sq[:],
                                        axis=mybir.AxisListType.X, op=OP.add)
                tv = wpool.tile([128, 1], mybir.dt.float32, tag="ln4",
                                bufs=2, name="tv")
                nc.vector.tensor_scalar(out=tv[:], in0=ss[:],
                                        scalar1=1.0 / D, scalar2=LN_EPS,
                                        op0=OP.mult, op1=OP.add)
                rinv = wpool.tile([128, 1], mybir.dt.float32, tag="ln5",
                                  bufs=2, name="rinv")
                nc.vector.reciprocal(rinv[:], tv[:])
                rstd = wpool.tile([128, 1], mybir.dt.float32, tag="ln6",
                                  bufs=2, name="rstd")
                nc.scalar.activation(rstd[:], rinv[:], AF.Sqrt)
                ot = wpool.tile([128, D], mybir.dt.float32, tag="ot",
                                bufs=3, name="ot")
                nc.scalar.activation(ot[:], yc[:], AF.Gelu, scale=rstd[:])
                nc.sync.dma_start(y_out[t * 128:(t + 1) * 128, :], ot[:])

    nc.compile()
    return nc


def kernel(x, rows, cols, vals, W, b, ws_seq_0, ws_seq_1, ws_res_0,
           ws_res_1):
    in_maps, T, padded_id = preprocess(x, rows, cols, vals)
    weights = make_weights(ws_seq_0, ws_seq_1, ws_res_0, ws_res_1)
    nc = build_program(T, weights, mode=MODE)

    bb = np.tile(np.asarray(b, dtype=np.float32)[None, :], (128, 1))
    iota_np = np.tile(np.arange(128, dtype=np.float32)[None, :], (128, 1))
    W_np = np.asarray(W, dtype=np.float32)
    for m in in_maps:
        m["W_in"] = W_np
        m["bb_in"] = bb
        m["iota_in"] = iota_np

    LAST_BUILD.clear()
    LAST_BUILD.update({"nc": nc, "in_maps": in_maps, "T": T})

    res = run_bass_kernel_spmd(nc, in_maps, core_ids=list(range(N_CORES)))
    y_all = np.concatenate(
        [res.results[c]["y_out"] for c in range(N_CORES)], axis=0)
    return y_all[padded_id].astype(np.float32)

